# revision 4
# baseline (speedup 1.0000x reference)
"""Mixtral GQA attention (B=2, S=2048, H=4096, 32 q heads / 8 kv heads,
interleaved RoPE, causal; sliding window 4096 >= S so it is plain causal)
on 8 Trainium2 NeuronCores.

Sharding: DP=2 over batch x TP=4 over kv-head pairs. Core c = 4*b + t
handles batch b, kv heads {2t, 2t+1}, q heads [8t, 8t+8). Each core
computes qkv projection (transposed layout), RoPE, attention, and its
partial of the wo projection; the host sums the 4 partials per batch.

Device layout notes:
 - Everything is computed transposed ([feature, token]) so the PE
   contraction dim always sits on partitions; no on-device transposes
   are needed except V (32 small PE transposes).
 - RoPE is applied neox-style: the wq/wk columns are permuted on the
   host (even dims then odd dims) which turns GPT-J interleaved rotary
   into contiguous half rotations. q.k dot products are invariant.
 - Matmuls run in float32r (fp32 truncated to ~FP22, full PE rate at
   moving-dim >= 256): ~1.5e-4 relative error.
 - softmax skips the max-subtraction (scores are O(10) here), masks the
   upper triangle with affine_select after exp, reduces over keys
   (partition dim) with a ones-vector matmul, and normalizes via a
   K=1 broadcast matmul of the reciprocal row sums.
"""

import sys

sys.path.insert(0, "/opt/trn_rl_repo")

import numpy as np

import concourse.bass as bass  # noqa: F401
import concourse.mybir as mybir
import concourse.tile as tile
from concourse import bacc
from concourse.bass_utils import run_bass_kernel_spmd

F32 = mybir.dt.float32
F32R = mybir.dt.float32r

B = 2
S = 2048
H = 4096
NH = 32
NKV = 8
HD = 128
GROUP = NH // NKV
ROPE_BASE = 10000.0
SCALE = HD**-0.5

N_CORES = 8
TP = 4  # kv-head-pair groups
Q_PER_CORE = 8
KV_PER_CORE = 2

NC_BLK = Q_PER_CORE + 2 * KV_PER_CORE  # 12 feature blocks of 128 in stage 1
NSEG = 4  # contraction (H) segments
HB = H // 128 // NSEG  # h-blocks per segment = 8
TCH = 4  # token chunks
TC_W = S // TCH  # 512
SB = S // 128  # 16 key blocks

_compiled = None


def _build():
    nc = bacc.Bacc("TRN2", target_bir_lowering=False, debug=False,
                   num_devices=N_CORES)

    hid_t = nc.declare_dram_parameter("hid_t", [H, S], F32, isOutput=False)
    w12 = nc.declare_dram_parameter("w12", [H, NC_BLK * 128], F32, isOutput=False)
    wo = nc.declare_dram_parameter("wo", [Q_PER_CORE * 128, H], F32, isOutput=False)
    cos2 = nc.declare_dram_parameter("cos2", [128, S], F32, isOutput=False)
    sinpm = nc.declare_dram_parameter("sinpm", [128, S], F32, isOutput=False)
    identd = nc.declare_dram_parameter("identd", [128, 128], F32, isOutput=False)
    onesrd = nc.declare_dram_parameter("onesrd", [1, 128], F32, isOutput=False)
    onescd = nc.declare_dram_parameter("onescd", [128, 1], F32, isOutput=False)
    out = nc.declare_dram_parameter("out", [S, H], F32, isOutput=True)

    with tile.TileContext(nc) as tc:
        with tc.tile_pool(name="consts", bufs=1) as consts, \
             tc.tile_pool(name="acc", bufs=1) as accp:
            ident = consts.tile([128, 128], F32R, name="ident", tag="ident")
            nc.sync.dma_start(out=ident[:], in_=identd[:].bitcast(F32R))
            ones_r = consts.tile([1, 128], F32R, name="ones_r", tag="ones_r")
            nc.sync.dma_start(out=ones_r[:], in_=onesrd[:].bitcast(F32R))
            ones_c = consts.tile([128, 1], F32R, name="ones_c", tag="ones_c")
            nc.sync.dma_start(out=ones_c[:], in_=onescd[:].bitcast(F32R))
            cost = consts.tile([128, S], F32, name="cost", tag="cost")
            nc.sync.dma_start(out=cost[:], in_=cos2[:])
            sint = consts.tile([128, S], F32, name="sint", tag="sint")
            nc.sync.dma_start(out=sint[:], in_=sinpm[:])

            acc = [accp.tile([128, S], F32R, name=f"acc{c}", tag=f"acc{c}")
                   for c in range(NC_BLK)]

            # ---- stage 1: qkv^T = w12^T @ hid_t over 4 H-segments
            with tc.tile_pool(name="wseg", bufs=NC_BLK + 1) as wp, \
                 tc.tile_pool(name="hidt", bufs=12) as hp, \
                 tc.tile_pool(name="ps1", bufs=6, space="PSUM") as ps1:
                for seg in range(NSEG):
                    wt = []
                    for c in range(NC_BLK):
                        w_tile = wp.tile([128, HB, 128], F32R,
                                         name=f"w_{seg}_{c}", tag="w")
                        nc.sync.dma_start(
                            out=w_tile[:],
                            in_=w12[seg * HB * 128:(seg + 1) * HB * 128,
                                    c * 128:(c + 1) * 128]
                            .rearrange("(hb p) c -> p hb c", p=128)
                            .bitcast(F32R),
                        )
                        wt.append(w_tile)
                    for t in range(TCH):
                        ht = []
                        for hb in range(HB):
                            h_tile = hp.tile([128, TC_W], F32R,
                                             name=f"h_{seg}_{t}_{hb}", tag="h")
                            nc.sync.dma_start(
                                out=h_tile[:],
                                in_=hid_t[(seg * HB + hb) * 128:
                                          (seg * HB + hb + 1) * 128,
                                          t * TC_W:(t + 1) * TC_W].bitcast(F32R),
                            )
                            ht.append(h_tile)
                        for c in range(NC_BLK):
                            pt = ps1.tile([128, TC_W], F32,
                                          name=f"p1_{seg}_{t}_{c}", tag="ps1")
                            for hb in range(HB):
                                nc.tensor.matmul(pt[:], wt[c][:, hb, :], ht[hb][:],
                                                 start=(hb == 0),
                                                 stop=(hb == HB - 1))
                            dst = acc[c][:, t * TC_W:(t + 1) * TC_W]
                            if seg == 0:
                                nc.vector.tensor_copy(dst, pt[:])
                            else:
                                nc.vector.tensor_add(dst, dst, pt[:])

            # ---- RoPE on q (blocks 0..7) and k (blocks 8..9), in place
            with tc.tile_pool(name="rope", bufs=2) as rp:
                for c in range(Q_PER_CORE + KV_PER_CORE):
                    blk = acc[c]
                    tmp = rp.tile([128, S], F32, name=f"ropetmp{c}", tag="ropetmp")
                    nc.vector.tensor_copy(tmp[0:64, :], blk[64:128, :])
                    nc.vector.tensor_copy(tmp[64:128, :], blk[0:64, :])
                    nc.vector.tensor_mul(tmp[:], tmp[:], sint[:])
                    nc.vector.tensor_mul(blk[:], blk[:], cost[:])
                    nc.vector.tensor_add(blk[:], blk[:], tmp[:])

            with tc.tile_pool(name="vnat", bufs=1) as vp:
                # ---- stage 2: V natural layout via PE transposes
                vnat = [None] * (KV_PER_CORE * SB)
                with tc.tile_pool(name="ps2", bufs=2, space="PSUM") as ps2:
                    for kv in range(KV_PER_CORE):
                        vt = acc[Q_PER_CORE + KV_PER_CORE + kv]
                        for sb in range(SB):
                            ptt = ps2.tile([128, 128], F32R,
                                           name=f"pt2_{kv}_{sb}", tag="ps2")
                            nc.tensor.transpose(
                                ptt[:],
                                vt[:, sb * 128:(sb + 1) * 128],
                                ident[:],
                            )
                            vtile = vp.tile([128, 128], F32R,
                                            name=f"v{kv}_{sb}", tag=f"v{kv}_{sb}")
                            nc.vector.tensor_copy(vtile[:], ptt[:])
                            vnat[kv * SB + sb] = vtile

                # ---- stage 3: attention per q head; attn overwrites acc[g]
                with tc.tile_pool(name="probs", bufs=6) as pp, \
                     tc.tile_pool(name="recip", bufs=2) as rcp, \
                     tc.tile_pool(name="ps_s", bufs=2, space="PSUM") as ps_s, \
                     tc.tile_pool(name="ps_pv", bufs=2, space="PSUM") as ps_pv, \
                     tc.tile_pool(name="ps_sum", bufs=1, space="PSUM") as ps_sm, \
                     tc.tile_pool(name="ps_bc", bufs=1, space="PSUM") as ps_bc:
                    for g in range(Q_PER_CORE):
                        kv = g // GROUP
                        kt = acc[Q_PER_CORE + kv]
                        for t in range(TCH):
                            nsb = 4 * t + 4  # key blocks 0 .. 4t+3
                            pv = ps_pv.tile([128, TC_W], F32,
                                            name=f"pv_{g}_{t}", tag="pv")
                            sm = ps_sm.tile([1, TC_W], F32,
                                            name=f"sm_{g}_{t}", tag="sum")
                            for sb in range(nsb):
                                sc = ps_s.tile([128, TC_W], F32,
                                               name=f"sc_{g}_{t}_{sb}", tag="s")
                                nc.tensor.matmul(
                                    sc[:],
                                    kt[:, sb * 128:(sb + 1) * 128],
                                    acc[g][:, t * TC_W:(t + 1) * TC_W],
                                    start=True, stop=True,
                                )
                                pr = pp.tile([128, TC_W], F32R,
                                             name=f"pr_{g}_{t}_{sb}", tag="pr")
                                nc.scalar.activation(
                                    pr[:], sc[:],
                                    mybir.ActivationFunctionType.Exp)
                                j = sb - 4 * t
                                if j >= 0:
                                    # zero where key > query
                                    nc.gpsimd.affine_select(
                                        out=pr[:], in_=pr[:],
                                        compare_op=mybir.AluOpType.is_ge,
                                        fill=0.0, base=-128 * j,
                                        pattern=[[1, TC_W]],
                                        channel_multiplier=-1,
                                    )
                                nc.tensor.matmul(pv[:], vnat[kv * SB + sb][:],
                                                 pr[:], start=(sb == 0),
                                                 stop=(sb == nsb - 1))
                                nc.tensor.matmul(sm[:], ones_c[:], pr[:],
                                                 start=(sb == 0),
                                                 stop=(sb == nsb - 1))
                            rc = rcp.tile([1, TC_W], F32R,
                                          name=f"rc_{g}_{t}", tag="rc")
                            with nc.allow_low_precision("softmax denom recip"):
                                nc.vector.reciprocal(rc[:], sm[:])
                            bc = ps_bc.tile([128, TC_W], F32,
                                            name=f"bc_{g}_{t}", tag="bc")
                            nc.tensor.matmul(bc[:], ones_r[:], rc[:],
                                             start=True, stop=True)
                            dst = acc[g][:, t * TC_W:(t + 1) * TC_W]
                            nc.scalar.copy(dst, pv[:])
                            nc.vector.tensor_mul(dst, dst, bc[:])

            # ---- stage 4: out[t, n] = sum_g attn_g^T @ wo_g
            with tc.tile_pool(name="wop", bufs=2) as wops, \
                 tc.tile_pool(name="outp", bufs=4) as op, \
                 tc.tile_pool(name="ps4", bufs=4, space="PSUM") as ps4:
                for n in range(H // TC_W):
                    wn = wops.tile([128, Q_PER_CORE, TC_W], F32R,
                                   name=f"wo_{n}", tag="wo")
                    nc.sync.dma_start(
                        out=wn[:],
                        in_=wo[:, n * TC_W:(n + 1) * TC_W]
                        .rearrange("(g p) c -> p g c", p=128)
                        .bitcast(F32R),
                    )
                    for tb in range(SB):
                        po = ps4.tile([128, TC_W], F32,
                                      name=f"po_{n}_{tb}", tag="po")
                        for g in range(Q_PER_CORE):
                            nc.tensor.matmul(
                                po[:],
                                acc[g][:, tb * 128:(tb + 1) * 128],
                                wn[:, g, :],
                                start=(g == 0), stop=(g == Q_PER_CORE - 1),
                            )
                        ot = op.tile([128, TC_W], F32,
                                     name=f"ot_{n}_{tb}", tag="ot")
                        nc.scalar.copy(ot[:], po[:])
                        nc.sync.dma_start(
                            out=out[tb * 128:(tb + 1) * 128,
                                    n * TC_W:(n + 1) * TC_W],
                            in_=ot[:],
                        )

    nc.compile()
    return nc


def _get_compiled():
    global _compiled
    if _compiled is None:
        _compiled = _build()
    return _compiled


_EVEN_ODD = np.concatenate([np.arange(0, HD, 2), np.arange(1, HD, 2)])


def _prep_core_inputs(hidden_states, positions, wqkv, wo):
    """Returns list of 8 in_maps (core c = 4*b + t)."""
    inv_freq = ROPE_BASE ** (-np.arange(0, HD, 2, dtype=np.float32) / HD)
    ident = np.eye(128, dtype=np.float32)
    ones_r = np.ones((1, 128), dtype=np.float32)
    ones_c = np.ones((128, 1), dtype=np.float32)

    per_batch = []
    for b in range(B):
        hid_t = np.ascontiguousarray(hidden_states[b].T.astype(np.float32))
        ang = positions[b].astype(np.float32)[:, None] * inv_freq[None, :]
        cos = np.cos(ang).T.astype(np.float32)  # [64, S]
        sin = np.sin(ang).T.astype(np.float32)
        cos2 = np.ascontiguousarray(np.concatenate([cos, cos], axis=0))
        sinpm = np.ascontiguousarray(np.concatenate([-sin, sin], axis=0))
        per_batch.append((hid_t, cos2, sinpm))

    in_maps = []
    for c in range(N_CORES):
        b, t = c // TP, c % TP
        hid_t, cos2, sinpm = per_batch[b]
        blocks = []
        for gh in range(Q_PER_CORE):  # q heads, permuted + pre-scaled
            h = Q_PER_CORE * t + gh
            blocks.append(wqkv[:, h * HD:(h + 1) * HD][:, _EVEN_ODD] * SCALE)
        for m in range(KV_PER_CORE):  # k heads, permuted
            h = KV_PER_CORE * t + m
            blocks.append(
                wqkv[:, NH * HD + h * HD: NH * HD + (h + 1) * HD][:, _EVEN_ODD])
        for m in range(KV_PER_CORE):  # v heads, natural
            h = KV_PER_CORE * t + m
            base = (NH + NKV) * HD
            blocks.append(wqkv[:, base + h * HD: base + (h + 1) * HD])
        w12 = np.ascontiguousarray(
            np.concatenate(blocks, axis=1).astype(np.float32))
        wo_shard = np.ascontiguousarray(
            wo[Q_PER_CORE * HD * t: Q_PER_CORE * HD * (t + 1), :]
            .astype(np.float32))
        in_maps.append({
            "hid_t": hid_t, "w12": w12, "wo": wo_shard,
            "cos2": cos2, "sinpm": sinpm,
            "identd": ident, "onesrd": ones_r, "onescd": ones_c,
        })
    return in_maps


def kernel(hidden_states, positions, wqkv, wo):
    hidden_states = np.asarray(hidden_states)
    positions = np.asarray(positions)
    wqkv = np.asarray(wqkv)
    wo = np.asarray(wo)
    nc = _get_compiled()
    in_maps = _prep_core_inputs(hidden_states, positions, wqkv, wo)
    res = run_bass_kernel_spmd(nc, in_maps, list(range(N_CORES)))
    full = np.zeros((B, S, H), dtype=np.float32)
    for c in range(N_CORES):
        full[c // TP] += res.results[c]["out"]
    return full
